# revision 11
# baseline (speedup 1.0000x reference)
"""Self-contained TRN2 Bass kernel for the GCN message-passing problem.

8-core SPMD, v3: batched SWDGE dma_gather for the per-edge message gathers
(the v2 per-128-edge indirect_dma_start paid ~1us SWDGE fixed overhead per
instruction on the Pool engine -> ~5ms serialized; dma_gather amortizes it
over thousands of rows), bf16 message table with 256B rows, bf16 one-hot
selection matmuls (full-rate PE), self-loops as structured sequential-DMA
tiles.

Design:
- Nodes sharded by dst across cores (NS = N/C contiguous nodes per core).
- Gather table t[v] = dinv[v]*h[v] stored bf16 as [N, 128] (cols 0:64 are
  values, 64:128 junk padding so rows are 256B -- the dma_gather minimum
  elem stride). Replicated in every core's DRAM via AllGather each layer.
- dma_gather uses int16 indices -> the table is windowed into 4 buckets of
  <=32768 rows; edges are grouped per (4-block supergroup, bucket), sorted
  by dst block, each (block,bucket) run padded to a multiple of 128. One
  dma_gather per (group,bucket) lands rows as [128, nch, 128] tiles whose
  128-chunks each feed one one-hot matmul into that block's PSUM.
- One-hot selection (bf16): is_equal(iota, dstloc) * dinv[dst] on DVE, then
  PE psum[64f,128d] += g[128e,:64].T @ oh[128e,128d].
- Self-loop term per block from bounce (own shard, sequential DMA).
- agg kept feature-major [64, NS]; weight matmuls fp32; bias+relu on ACT.
- Pooling: one-hot over G graphs into psum [64, G], AllReduce, tiny MLP.
"""
import time

import numpy as np
import ml_dtypes
import jax
from jax.sharding import Mesh, PartitionSpec
from jax.experimental.shard_map import shard_map

from concourse import bass2jax
from concourse.bass2jax import _bass_exec_p, install_neuronx_cc_hook

from dataclasses import dataclass

import concourse.bass as bass
import concourse.bacc as bacc
import concourse.mybir as mybir
import concourse.tile as tile
from concourse import library_config

F32 = mybir.dt.float32
BF16 = mybir.dt.bfloat16
I16 = mybir.dt.int16

BF = ml_dtypes.bfloat16


@dataclass
class Meta:
    N: int
    F: int
    H: int
    G: int
    L: int
    C: int
    NS: int
    NB: int
    NK: int
    NG: int
    SB: int
    kbase: tuple
    klen: tuple
    nchgk: tuple      # [NG][NK] chunks per (group,bucket)
    subs: tuple       # [NG][NK] tuple of (icol, nch_sub, t_start) sub-gathers
    blk_ops: tuple    # [NB] tuple of (k, cc, t) chunk ops
    T_tot: int
    ICOLS: int
    CH: int = 512


def preprocess(x, edge_index, batch, W_emb, b_emb, conv_W, conv_b,
               W1, b1, W2, b2, W3, b3, n_cores=8, G=None):
    """Host-side index preprocessing. Returns (meta, in_maps)."""
    x = np.asarray(x, np.float32)
    ei = np.asarray(edge_index, np.int64)
    batch = np.asarray(batch, np.int64)
    N, F = x.shape
    H = int(np.asarray(W_emb).shape[1])
    L = int(np.asarray(conv_W).shape[0])
    C = n_cores
    assert N % C == 0
    NS = N // C
    NB = (NS + 127) // 128
    if G is None:
        G = int(batch.max()) + 1 if batch.size else 1

    BK = 32768
    NK = (N + BK - 1) // BK
    kbase = tuple(k * BK for k in range(NK))
    klen = tuple(min(BK, N - k * BK) for k in range(NK))
    SB = 4
    NG = (NB + SB - 1) // SB

    loop = np.arange(N, dtype=np.int64)
    deg = (np.bincount(np.concatenate([ei[1], loop]), minlength=N)
           .astype(np.float64))
    dinv = (1.0 / np.sqrt(np.maximum(deg, 1.0))).astype(np.float32)
    src, dst = ei[0], ei[1]

    core = dst // NS
    block = (dst % NS) // 128
    bucket = src // BK
    group = block // SB
    # order: core, group, bucket, block
    order = np.lexsort((block, bucket, group, core))
    src_s, dst_s = src[order], dst[order]
    core_s = core[order]
    block_s = block[order]
    bucket_s = bucket[order]

    # counts per (core, block, bucket)
    cnt = np.zeros((C, NB, NK), np.int64)
    np.add.at(cnt, (core_s, block_s, bucket_s), 1)
    # common chunk counts per (block, bucket): max over cores
    nch_bk = ((cnt.max(axis=0) + 127) // 128).astype(np.int64)  # [NB, NK]

    # global chunk numbering: g-major, then k, then blocks in g, then local
    t0_bk = np.zeros((NB, NK), np.int64)
    nchgk = np.zeros((NG, NK), np.int64)
    t = 0
    for g in range(NG):
        blks = range(g * SB, min((g + 1) * SB, NB))
        for k in range(NK):
            for b in blks:
                t0_bk[b, k] = t
                t += int(nch_bk[b, k])
            nchgk[g, k] = sum(int(nch_bk[b, k]) for b in blks)
    T_tot = t

    icol0 = np.zeros((NG, NK), np.int64)
    ic = 0
    for g in range(NG):
        for k in range(NK):
            icol0[g, k] = ic
            ic += int(nchgk[g, k]) * 8   # 128 idx/chunk / 16 per column
    ICOLS = ic

    # Sub-gathers: each (g,k) chunk run is cut into <=SUBCH-chunk gathers
    # (single_packet mode wedges past 1024 ring descriptors = 8 chunks).
    # subs[g][k] = ((icol, nch_sub, t_start), ...)
    SUBCH = 8
    subs = []
    for g in range(NG):
        row = []
        for k in range(NK):
            nch = int(nchgk[g, k])
            lst = []
            blks = list(range(g * SB, min((g + 1) * SB, NB)))
            tstart = int(t0_bk[blks[0], k]) if nch else 0
            for j0 in range(0, nch, SUBCH):
                nsub = min(SUBCH, nch - j0)
                lst.append((int(icol0[g, k]) + j0 * 8, nsub, tstart + j0))
            row.append(tuple(lst))
        subs.append(tuple(row))

    # per-block matmul op list (k, sub-index, chunk-offset-in-sub, global chunk)
    blk_ops = []
    for b in range(NB):
        g = b // SB
        blks = range(g * SB, min((g + 1) * SB, NB))
        ops = []
        for k in range(NK):
            cc0 = sum(int(nch_bk[bb, k]) for bb in blks if bb < b)
            for j in range(int(nch_bk[b, k])):
                cc = cc0 + j
                ops.append((k, cc // SUBCH, cc % SUBCH, int(t0_bk[b, k]) + j))
        blk_ops.append(tuple(ops))

    meta = Meta(N=N, F=F, H=H, G=G, L=L, C=C, NS=NS, NB=NB, NK=NK, NG=NG,
                SB=SB, kbase=kbase, klen=klen,
                nchgk=tuple(tuple(int(v) for v in row) for row in nchgk),
                subs=tuple(subs),
                blk_ops=tuple(blk_ops), T_tot=T_tot, ICOLS=ICOLS)

    # per-core start offsets into the sorted edge array for (block, bucket)
    starts = np.zeros(C * NB * NK, np.int64)
    cnt_cbk = np.transpose(cnt, (0, 1, 2))  # already [C, NB, NK]
    # lexsort order is core, group, bucket, block -- NOT core, block, bucket.
    # Build per-(c,g,k,b) segment starts by walking in that order.
    seg_sizes = np.zeros((C, NG, NK, SB), np.int64)
    for c in range(C):
        for g in range(NG):
            blks = list(range(g * SB, min((g + 1) * SB, NB)))
            for ki in range(NK):
                for bi, b in enumerate(blks):
                    seg_sizes[c, g, ki, bi] = cnt[c, b, ki]
    flat_sizes = seg_sizes.ravel()
    flat_starts = np.zeros_like(flat_sizes)
    flat_starts[1:] = np.cumsum(flat_sizes)[:-1]
    seg_starts = flat_starts.reshape(C, NG, NK, SB)

    xT = np.ascontiguousarray(x.T)  # [F, N] f32

    iota128 = np.tile(np.arange(128, dtype=np.float32), (128, 1))
    iota_part = np.arange(128, dtype=np.float32).reshape(128, 1)
    iotaG = np.tile(np.arange(G, dtype=np.float32), (128, 1))
    ident_f = np.eye(128, dtype=np.float32)

    cntg = np.bincount(batch, minlength=G).astype(np.float32)
    invc = np.tile((1.0 / np.maximum(cntg, 1.0))[None, :], (H, 1)).astype(np.float32)

    conv_W = np.asarray(conv_W, np.float32)
    conv_b = np.asarray(conv_b, np.float32)

    in_maps = []
    for c in range(C):
        base = c * NS
        idx_img = np.zeros((128, ICOLS), np.int16)
        dstloc = np.full((128, T_tot), -1.0, np.float32)
        dinvdst = np.zeros((128, T_tot), np.float32)
        for g in range(NG):
            blks = list(range(g * SB, min((g + 1) * SB, NB)))
            for k in range(NK):
                nch = int(nchgk[g, k])
                if nch == 0:
                    continue
                nidx = nch * 128
                srel = np.zeros(nidx, np.int64)
                dl = np.full(nidx, -1.0, np.float32)
                dw = np.zeros(nidx, np.float32)
                pos = 0
                for bi, b in enumerate(blks):
                    n = int(cnt[c, b, k])
                    s0 = int(seg_starts[c, g, k, bi])
                    if n > 0:
                        e_src = src_s[s0:s0 + n]
                        e_dst = dst_s[s0:s0 + n]
                        srel[pos:pos + n] = e_src - kbase[k]
                        dl[pos:pos + n] = (e_dst - base - b * 128)
                        dw[pos:pos + n] = dinv[e_dst]
                    pos0 = pos
                    pos += int(nch_bk[b, k]) * 128
                    # pad rows within this block's chunk range gather row 0
                    # (valid; one-hot row is all-zero via dstloc=-1)
                assert pos <= nidx
                assert pos == nidx, (pos, nidx)
                # idx image: j -> [p = j%16 (replicated x8), col = j//16]
                arr = srel.reshape(-1, 16).T.astype(np.int16)  # [16, ncol]
                ic0 = int(icol0[g, k])
                idx_img[:, ic0:ic0 + nch * 8] = np.tile(arr, (8, 1))
                # dstloc/dinvdst: chunk-major columns
                t0 = None
                # global chunks for this (g,k) span consecutive t from
                # t0_bk[blks[0], k] in block order
                dl2 = dl.reshape(nch, 128).T      # [128, nch]
                dw2 = dw.reshape(nch, 128).T
                tstart = int(t0_bk[blks[0], k])
                dstloc[:, tstart:tstart + nch] = dl2
                dinvdst[:, tstart:tstart + nch] = dw2

        dinv_sh = dinv[base:base + NS]
        dinv_nm = np.zeros((128, NB), np.float32)
        poolid = np.full((128, NB), -1.0, np.float32)
        for b in range(NB):
            w = min(128, NS - b * 128)
            dinv_nm[:w, b] = dinv_sh[b * 128:b * 128 + w]
            poolid[:w, b] = batch[base + b * 128: base + b * 128 + w]
        m = {
            "x_t": np.ascontiguousarray(xT[:, base:base + NS]).astype(BF),
            "idx_img": idx_img,
            "dstloc": dstloc,
            "dinvdst": dinvdst,
            "poolid": poolid,
            "dinv_nm": dinv_nm,
            "iota128": iota128.astype(BF),
            "iota_part": iota_part,
            "iotag": iotaG.astype(BF),
            "ident_f": ident_f,
            "wemb": np.asarray(W_emb, np.float32).astype(BF),
            "bemb": np.asarray(b_emb, np.float32).reshape(H, 1),
            "invc": invc,
            "w1": np.asarray(W1, np.float32),
            "b1": np.asarray(b1, np.float32).reshape(-1, 1),
            "w2": np.asarray(W2, np.float32),
            "b2": np.asarray(b2, np.float32).reshape(-1, 1),
            "w3": np.asarray(W3, np.float32),
            "b3": np.asarray(b3, np.float32).reshape(1, 1),
        }
        for i in range(L):
            m[f"cw{i}"] = np.ascontiguousarray(conv_W[i])
            m[f"cb{i}"] = conv_b[i].reshape(H, 1)
        in_maps.append(m)
    return meta, in_maps


def build_nc(meta: Meta, gbufs=24, ohbufs=24, repeats=1,
             dbg_L=None, dbg_ng=None):
    N, F, H, G, L, C = meta.N, meta.F, meta.H, meta.G, meta.L, meta.C
    NS, NB, CH = meta.NS, meta.NB, meta.CH
    NK, NG, SB = meta.NK, meta.NG, meta.SB
    kbase, klen = meta.kbase, meta.klen
    nchgk, subs, blk_ops = meta.nchgk, meta.subs, meta.blk_ops
    T_tot, ICOLS = meta.T_tot, meta.ICOLS
    NCH = (NS + CH - 1) // CH

    nc = bacc.Bacc("TRN2", target_bir_lowering=False, debug=False, num_devices=C,
                   num_swdge_queues=4)

    def EIN(name, shape, dt):
        return nc.dram_tensor(name, list(shape), dt, kind="ExternalInput")

    x_t = EIN("x_t", [F, NS], BF16)
    idx_img = EIN("idx_img", [128, ICOLS], I16)
    dstloc = EIN("dstloc", [128, T_tot], F32)
    dinvdst = EIN("dinvdst", [128, T_tot], F32)
    poolid = EIN("poolid", [128, NB], F32)
    dinv_nm = EIN("dinv_nm", [128, NB], F32)
    iota128 = EIN("iota128", [128, 128], BF16)
    iota_part = EIN("iota_part", [128, 1], F32)
    iotag = EIN("iotag", [128, G], BF16)
    ident_f = EIN("ident_f", [128, 128], F32)
    wemb = EIN("wemb", [F, H], BF16)
    bemb = EIN("bemb", [H, 1], F32)
    invc = EIN("invc", [H, G], F32)
    w1 = EIN("w1", [H, H], F32)
    b1 = EIN("b1", [H, 1], F32)
    w2 = EIN("w2", [H, H // 2], F32)
    b2 = EIN("b2", [H // 2, 1], F32)
    w3 = EIN("w3", [H // 2, 1], F32)
    b3 = EIN("b3", [1, 1], F32)
    cw = [EIN(f"cw{i}", [H, H], F32) for i in range(L)]
    cb = [EIN(f"cb{i}", [H, 1], F32) for i in range(L)]

    out_d = nc.dram_tensor("out", [1, G], F32, kind="ExternalOutput")

    table_a = nc.dram_tensor("table_a", [N, 128], BF16, addr_space="Shared")
    table_b = nc.dram_tensor("table_b", [N, 128], BF16, addr_space="Shared")
    bounce = nc.dram_tensor("bounce", [NS, 128], BF16)
    pool_in = nc.dram_tensor("pool_in", [H, G], F32)
    pool_out = nc.dram_tensor("pool_out", [H, G], F32, addr_space="Shared")

    groups = [list(range(C))]

    with tile.TileContext(nc) as tc:
        import contextlib
        ctx = contextlib.ExitStack()
        with ctx:
            P = ctx.enter_context
            persist = P(tc.tile_pool(name="persist", bufs=1))
            xpool = P(tc.tile_pool(name="xpool", bufs=3))
            gpool = P(tc.tile_pool(name="gpool", bufs=gbufs))
            ohpool = P(tc.tile_pool(name="ohpool", bufs=ohbufs))
            slpool = P(tc.tile_pool(name="slpool", bufs=4))
            stpool = P(tc.tile_pool(name="stpool", bufs=4))
            pohpool = P(tc.tile_pool(name="pohpool", bufs=3))
            bp_ps = P(tc.tile_pool(name="bp_ps", bufs=3, space="PSUM"))
            mm_ps = P(tc.tile_pool(name="mm_ps", bufs=2, space="PSUM"))
            tr_ps = P(tc.tile_pool(name="tr_ps", bufs=2, space="PSUM"))

            nc.gpsimd.load_library(library_config.mlp)

            def load(name, ap, shape, dt):
                t = persist.tile(list(shape), dt, tag=name, name=name)
                nc.sync.dma_start(out=t[:], in_=ap[:])
                return t

            idx_sb = load("idx_sb", idx_img, [128, ICOLS], I16)
            dstloc_sb = load("dstloc_sb", dstloc, [128, T_tot], F32)
            dinvdst_sb = load("dinvdst_sb", dinvdst, [128, T_tot], F32)
            poolid_sb = load("poolid_sb", poolid, [128, NB], F32)
            dinvnm_sb = load("dinvnm_sb", dinv_nm, [128, NB], F32)
            iota_sb = load("iota_sb", iota128, [128, 128], BF16)
            iotap_sb = load("iotap_sb", iota_part, [128, 1], F32)
            iotag_sb = load("iotag_sb", iotag, [128, G], BF16)
            identf_sb = load("identf_sb", ident_f, [128, 128], F32)
            wemb_sb = load("wemb_sb", wemb, [F, H], BF16)
            bemb_sb = load("bemb_sb", bemb, [H, 1], F32)
            invc_sb = load("invc_sb", invc, [H, G], F32)
            w1_sb = load("w1_sb", w1, [H, H], F32)
            b1_sb = load("b1_sb", b1, [H, 1], F32)
            w2_sb = load("w2_sb", w2, [H, H // 2], F32)
            b2_sb = load("b2_sb", b2, [H // 2, 1], F32)
            w3_sb = load("w3_sb", w3, [H // 2, 1], F32)
            b3_sb = load("b3_sb", b3, [1, 1], F32)
            cw_sb = [load(f"cw{i}_sb", cw[i], [H, H], F32) for i in range(L)]
            cb_sb = [load(f"cb{i}_sb", cb[i], [H, 1], F32) for i in range(L)]

            hagg = persist.tile([64, NS], F32, tag="hagg")
            h3n = persist.tile([128, NB * 64], BF16, tag="h3n")

            def chunks():
                for ci in range(NCH):
                    c0 = ci * CH
                    yield c0, min(CH, NS - c0)

            def table_write(table_out):
                for b in range(NB):
                    w = min(128, NS - b * 128)
                    tp = tr_ps.tile([128, 64], F32, tag="trp")
                    nc.tensor.transpose(
                        out=tp[:w, :], in_=hagg[:, b * 128:b * 128 + w],
                        identity=identf_sb[:64, :64])
                    st = stpool.tile([128, 128], BF16, tag="st")
                    nc.vector.tensor_scalar(
                        out=st[:w, :64], in0=tp[:w, :],
                        scalar1=dinvnm_sb[:w, b:b + 1], scalar2=None,
                        op0=mybir.AluOpType.mult)
                    nc.sync.dma_start(
                        out=bounce[b * 128:b * 128 + w, :], in_=st[:w, :])
                nc.gpsimd.collective_compute(
                    "AllGather", mybir.AluOpType.bypass,
                    replica_groups=groups,
                    ins=[bounce[:]], outs=[table_out[:]])

            for _rep in range(repeats):
                # ================= embed =================
                for c0, cwd in chunks():
                    xt = xpool.tile([F, CH], BF16, tag="xt")
                    nc.sync.dma_start(out=xt[:, :cwd], in_=x_t[:, c0:c0 + cwd])
                    ps = mm_ps.tile([64, CH], F32, tag="mmps")
                    nc.tensor.matmul(out=ps[:, :cwd], lhsT=wemb_sb[:], rhs=xt[:, :cwd],
                                     start=True, stop=True)
                    nc.scalar.activation(out=hagg[:, c0:c0 + cwd], in_=ps[:, :cwd],
                                         func=mybir.ActivationFunctionType.Relu,
                                         bias=bemb_sb[:, 0:1])
                table_write(table_a)

                # ================= conv layers =================
                tables = [table_a, table_b, table_a]
                n_layers = L if dbg_L is None else dbg_L
                qrr = [0]
                for li in range(n_layers):
                    t_in = tables[li]
                    for g in range(NG if dbg_ng is None else dbg_ng):
                        gts = {}
                        for k in range(NK):
                            for j, (icol, nsub, _ts) in enumerate(subs[g][k]):
                                gt = gpool.tile([128, 8, 128], BF16, tag="g")
                                nidx = nsub * 128
                                nc.gpsimd.dma_gather(
                                    gt[:, :nsub, :],
                                    t_in[kbase[k]:kbase[k] + klen[k], :],
                                    idx_sb[:, icol:icol + nsub * 8],
                                    nidx, nidx, 128,
                                    # single-packet -> 1 ring desc per row;
                                    # cap 1024/instr (ring size), spread over
                                    # all 4 SWDGE queues for DMA parallelism
                                    single_packet=True,
                                    queue_num=qrr[0] & 3)
                                qrr[0] += 1
                                gts[(k, j)] = gt
                        for b in range(g * SB, min((g + 1) * SB, NB)):
                            w = min(128, NS - b * 128)
                            ps = bp_ps.tile([64, 128], F32, tag="bps")
                            first = True
                            for (k, j, cc, t) in blk_ops[b]:
                                oh = ohpool.tile([128, 128], BF16, tag="oh")
                                nc.vector.tensor_scalar(
                                    out=oh[:], in0=iota_sb[:],
                                    scalar1=dstloc_sb[:, t:t + 1],
                                    scalar2=dinvdst_sb[:, t:t + 1],
                                    op0=mybir.AluOpType.is_equal,
                                    op1=mybir.AluOpType.mult)
                                nc.tensor.matmul(
                                    out=ps[:], lhsT=gts[(k, j)][:, cc, :64], rhs=oh[:],
                                    start=first, stop=False)
                                first = False
                            # self-loop tile from bounce (own shard rows)
                            gs = slpool.tile([128, 128], BF16, tag="gs")
                            nc.sync.dma_start(
                                out=gs[:w, :], in_=bounce[b * 128:b * 128 + w, :])
                            ohs = ohpool.tile([128, 128], BF16, tag="oh")
                            nc.vector.tensor_scalar(
                                out=ohs[:w, :], in0=iota_sb[:w, :],
                                scalar1=iotap_sb[:w, 0:1],
                                scalar2=dinvnm_sb[:w, b:b + 1],
                                op0=mybir.AluOpType.is_equal,
                                op1=mybir.AluOpType.mult)
                            nc.tensor.matmul(
                                out=ps[:], lhsT=gs[:w, :64], rhs=ohs[:w, :],
                                start=False, stop=True)
                            nc.vector.tensor_copy(
                                out=hagg[:, b * 128:b * 128 + w], in_=ps[:, :w])
                    for c0, cwd in chunks():
                        ps = mm_ps.tile([64, CH], F32, tag="mmps")
                        nc.tensor.matmul(out=ps[:, :cwd], lhsT=cw_sb[li][:],
                                         rhs=hagg[:, c0:c0 + cwd], start=True, stop=True)
                        nc.scalar.activation(out=hagg[:, c0:c0 + cwd], in_=ps[:, :cwd],
                                             func=mybir.ActivationFunctionType.Relu,
                                             bias=cb_sb[li][:, 0:1])
                    if li < L - 1:
                        table_write(tables[li + 1])

                # ================= pooling =================
                for b in range(NB):
                    w = min(128, NS - b * 128)
                    tp = tr_ps.tile([128, 64], F32, tag="trp")
                    nc.tensor.transpose(out=tp[:w, :], in_=hagg[:, b * 128:b * 128 + w],
                                        identity=identf_sb[:64, :64])
                    nc.vector.tensor_copy(out=h3n[:w, b * 64:(b + 1) * 64], in_=tp[:w, :])
                with tc.tile_pool(name="pool_ps", bufs=1, space="PSUM") as pool_ps:
                    pps = pool_ps.tile([64, G], F32, tag="pps")
                    for b in range(NB):
                        w = min(128, NS - b * 128)
                        ohp = pohpool.tile([128, G], BF16, tag="ohp")
                        nc.vector.tensor_scalar(
                            out=ohp[:w, :], in0=iotag_sb[:w, :],
                            scalar1=poolid_sb[:w, b:b + 1], scalar2=None,
                            op0=mybir.AluOpType.is_equal)
                        nc.tensor.matmul(out=pps[:], lhsT=h3n[:w, b * 64:(b + 1) * 64],
                                         rhs=ohp[:w, :], start=(b == 0), stop=(b == NB - 1))
                    psum_sb = persist.tile([64, G], F32, tag="psum_sb")
                    nc.vector.tensor_copy(out=psum_sb[:], in_=pps[:])
                nc.sync.dma_start(out=pool_in[:], in_=psum_sb[:])
                nc.gpsimd.collective_compute(
                    "AllReduce", mybir.AluOpType.add, replica_groups=groups,
                    ins=[pool_in[:]], outs=[pool_out[:]])
                pooled = persist.tile([64, G], F32, tag="pooled")
                nc.sync.dma_start(out=pooled[:], in_=pool_out[:])
                nc.vector.tensor_tensor(out=pooled[:], in0=pooled[:], in1=invc_sb[:],
                                        op=mybir.AluOpType.mult)
                # ================= MLP =================
                ps1 = mm_ps.tile([64, CH], F32, tag="mmps")
                nc.tensor.matmul(out=ps1[:, :G], lhsT=w1_sb[:], rhs=pooled[:],
                                 start=True, stop=True)
                r1 = persist.tile([64, G], F32, tag="r1")
                nc.scalar.activation(out=r1[:], in_=ps1[:64, :G],
                                     func=mybir.ActivationFunctionType.Relu,
                                     bias=b1_sb[:, 0:1])
                ps2 = mm_ps.tile([64, CH], F32, tag="mmps")
                nc.tensor.matmul(out=ps2[:32, :G], lhsT=w2_sb[:], rhs=r1[:],
                                 start=True, stop=True)
                r2 = persist.tile([32, G], F32, tag="r2")
                nc.scalar.activation(out=r2[:], in_=ps2[:32, :G],
                                     func=mybir.ActivationFunctionType.Relu,
                                     bias=b2_sb[:, 0:1])
                ps3 = mm_ps.tile([64, CH], F32, tag="mmps")
                nc.tensor.matmul(out=ps3[:1, :G], lhsT=w3_sb[:], rhs=r2[:],
                                 start=True, stop=True)
                outs = persist.tile([1, G], F32, tag="outs")
                nc.vector.tensor_scalar(out=outs[:], in0=ps3[:1, :G],
                                        scalar1=b3_sb[0:1, 0:1], scalar2=None,
                                        op0=mybir.AluOpType.add)
                nc.sync.dma_start(out=out_d[:], in_=outs[:])

    nc.compile()
    return nc


class SpmdRunner:
    def __init__(self, nc, n_cores):
        install_neuronx_cc_hook()
        self.nc = nc
        self.n_cores = n_cores
        partition_name = (nc.partition_id_tensor.name
                          if nc.partition_id_tensor else None)
        in_names, out_names, out_avals, zero_outs = [], [], [], []
        for alloc in nc.m.functions[0].allocations:
            if not isinstance(alloc, mybir.MemoryLocationSet):
                continue
            name = alloc.memorylocations[0].name
            if alloc.kind == "ExternalInput":
                if name != partition_name:
                    in_names.append(name)
            elif alloc.kind == "ExternalOutput":
                shape = tuple(alloc.tensor_shape)
                dt = mybir.dt.np(alloc.dtype)
                out_names.append(name)
                out_avals.append(jax.core.ShapedArray(shape, dt))
                zero_outs.append(np.zeros(shape, dt))
        self.in_names, self.out_names = in_names, out_names
        self.zero_outs = zero_outs
        bind_in_names = in_names + out_names
        if partition_name is not None:
            bind_in_names.append(partition_name)

        def _body(*args):
            operands = list(args)
            if partition_name is not None:
                operands.append(bass2jax.partition_id_tensor())
            outs = _bass_exec_p.bind(
                *operands,
                out_avals=tuple(out_avals),
                in_names=tuple(bind_in_names),
                out_names=tuple(out_names),
                lowering_input_output_aliases=(),
                sim_require_finite=False,
                sim_require_nnan=False,
                nc=nc,
            )
            return tuple(outs)

        devices = jax.devices()[:n_cores]
        self.mesh = Mesh(np.asarray(devices), ("core",))
        n_args = len(in_names) + len(zero_outs)
        in_specs = (PartitionSpec("core"),) * n_args
        out_specs = (PartitionSpec("core"),) * len(out_names)
        self.fn = jax.jit(
            shard_map(_body, mesh=self.mesh, in_specs=in_specs,
                      out_specs=out_specs, check_rep=False),
            keep_unused=True,
        )
        self._dev_in = None

    def set_inputs(self, in_maps):
        assert len(in_maps) == self.n_cores
        concat = [np.concatenate([np.asarray(in_maps[c][n])
                                  for c in range(self.n_cores)], axis=0)
                  for n in self.in_names]
        self._dev_in = [jax.device_put(a) for a in concat]
        self._dev_zeros = [
            jax.device_put(np.zeros((self.n_cores * z.shape[0], *z.shape[1:]),
                                    z.dtype)) for z in self.zero_outs]
        jax.block_until_ready(self._dev_in)

    def run(self):
        outs = self.fn(*self._dev_in, *self._dev_zeros)
        jax.block_until_ready(outs)
        return outs

    def results(self, outs):
        res = [dict() for _ in range(self.n_cores)]
        for i, name in enumerate(self.out_names):
            arr = np.asarray(outs[i])
            per = np.split(arr, self.n_cores, axis=0)
            for c in range(self.n_cores):
                res[c][name] = per[c]
        return res


_CACHE = {}


def _get_runner(meta, in_maps, repeats=1):
    key = (repeats, meta.T_tot, meta.ICOLS, meta.N, meta.L)
    if key not in _CACHE:
        nc = build_nc(meta, repeats=repeats)
        _CACHE[key] = SpmdRunner(nc, meta.C)
    return _CACHE[key]


def kernel(x, edge_index, batch, W_emb, b_emb, conv_W, conv_b,
           W1, b1, W2, b2, W3, b3):
    """Full (unsharded) inputs -> full [G, 1] float32 output."""
    G = 256
    meta, in_maps = preprocess(
        x, edge_index, batch, W_emb, b_emb, conv_W, conv_b,
        W1, b1, W2, b2, W3, b3, n_cores=8, G=G)
    r = _get_runner(meta, in_maps)
    r.set_inputs(in_maps)
    res = r.results(r.run())
    return np.ascontiguousarray(res[0]["out"].reshape(G, 1).astype(np.float32))


# revision 12
# speedup vs baseline: 1.9436x; 1.9436x over previous
"""Self-contained TRN2 Bass kernel for the GCN message-passing problem.

8-core SPMD, v3: batched SWDGE dma_gather for the per-edge message gathers
(the v2 per-128-edge indirect_dma_start paid ~1us SWDGE fixed overhead per
instruction on the Pool engine -> ~5ms serialized; dma_gather amortizes it
over thousands of rows), bf16 message table with 256B rows, bf16 one-hot
selection matmuls (full-rate PE), self-loops as structured sequential-DMA
tiles.

Design:
- Nodes sharded by dst across cores (NS = N/C contiguous nodes per core).
- Gather table t[v] = dinv[v]*h[v] stored bf16 as [N, 128] (cols 0:64 are
  values, 64:128 junk padding so rows are 256B -- the dma_gather minimum
  elem stride). Replicated in every core's DRAM via AllGather each layer.
- dma_gather uses int16 indices -> the table is windowed into 4 buckets of
  <=32768 rows; edges are grouped per (4-block supergroup, bucket), sorted
  by dst block, each (block,bucket) run padded to a multiple of 128. One
  dma_gather per (group,bucket) lands rows as [128, nch, 128] tiles whose
  128-chunks each feed one one-hot matmul into that block's PSUM.
- One-hot selection (bf16): is_equal(iota, dstloc) * dinv[dst] on DVE, then
  PE psum[64f,128d] += g[128e,:64].T @ oh[128e,128d].
- Self-loop term per block from bounce (own shard, sequential DMA).
- agg kept feature-major [64, NS]; weight matmuls fp32; bias+relu on ACT.
- Pooling: one-hot over G graphs into psum [64, G], AllReduce, tiny MLP.
"""
import time

import numpy as np
import ml_dtypes
import jax
from jax.sharding import Mesh, PartitionSpec
from jax.experimental.shard_map import shard_map

from concourse import bass2jax
from concourse.bass2jax import _bass_exec_p, install_neuronx_cc_hook

from dataclasses import dataclass

import concourse.bass as bass
import concourse.bacc as bacc
import concourse.mybir as mybir
import concourse.tile as tile
from concourse import library_config

F32 = mybir.dt.float32
BF16 = mybir.dt.bfloat16
I16 = mybir.dt.int16

BF = ml_dtypes.bfloat16


@dataclass
class Meta:
    N: int
    F: int
    H: int
    G: int
    L: int
    C: int
    NS: int
    NB: int
    NK: int
    NG: int
    SB: int
    kbase: tuple
    klen: tuple
    nchgk: tuple      # [NG][NK] chunks per (group,bucket)
    subs: tuple       # [NG][NK] tuple of (icol, nch_sub, t_start) sub-gathers
    blk_ops: tuple    # [NB] tuple of (k, cc, t) chunk ops
    T_tot: int
    ICOLS: int
    CH: int = 512


def preprocess(x, edge_index, batch, W_emb, b_emb, conv_W, conv_b,
               W1, b1, W2, b2, W3, b3, n_cores=8, G=None):
    """Host-side index preprocessing. Returns (meta, in_maps)."""
    x = np.asarray(x, np.float32)
    ei = np.asarray(edge_index, np.int64)
    batch = np.asarray(batch, np.int64)
    N, F = x.shape
    H = int(np.asarray(W_emb).shape[1])
    L = int(np.asarray(conv_W).shape[0])
    C = n_cores
    assert N % C == 0
    NS = N // C
    NB = (NS + 127) // 128
    if G is None:
        G = int(batch.max()) + 1 if batch.size else 1

    BK = 32768
    NK = (N + BK - 1) // BK
    kbase = tuple(k * BK for k in range(NK))
    klen = tuple(min(BK, N - k * BK) for k in range(NK))
    SB = 4
    NG = (NB + SB - 1) // SB

    loop = np.arange(N, dtype=np.int64)
    deg = (np.bincount(np.concatenate([ei[1], loop]), minlength=N)
           .astype(np.float64))
    dinv = (1.0 / np.sqrt(np.maximum(deg, 1.0))).astype(np.float32)
    src, dst = ei[0], ei[1]

    core = dst // NS
    block = (dst % NS) // 128
    bucket = src // BK
    group = block // SB
    # order: core, group, bucket, block
    order = np.lexsort((block, bucket, group, core))
    src_s, dst_s = src[order], dst[order]
    core_s = core[order]
    block_s = block[order]
    bucket_s = bucket[order]

    # counts per (core, block, bucket)
    cnt = np.zeros((C, NB, NK), np.int64)
    np.add.at(cnt, (core_s, block_s, bucket_s), 1)
    # common chunk counts per (block, bucket): max over cores
    nch_bk = ((cnt.max(axis=0) + 127) // 128).astype(np.int64)  # [NB, NK]

    # global chunk numbering: g-major, then k, then blocks in g, then local
    t0_bk = np.zeros((NB, NK), np.int64)
    nchgk = np.zeros((NG, NK), np.int64)
    t = 0
    for g in range(NG):
        blks = range(g * SB, min((g + 1) * SB, NB))
        for k in range(NK):
            for b in blks:
                t0_bk[b, k] = t
                t += int(nch_bk[b, k])
            nchgk[g, k] = sum(int(nch_bk[b, k]) for b in blks)
    T_tot = t

    icol0 = np.zeros((NG, NK), np.int64)
    ic = 0
    for g in range(NG):
        for k in range(NK):
            icol0[g, k] = ic
            ic += int(nchgk[g, k]) * 8   # 128 idx/chunk / 16 per column
    ICOLS = ic

    # Sub-gathers: each (g,k) chunk run is cut into <=SUBCH-chunk gathers
    # (single_packet mode wedges past 1024 ring descriptors = 8 chunks).
    # subs[g][k] = ((icol, nch_sub, t_start), ...)
    SUBCH = 24
    subs = []
    for g in range(NG):
        row = []
        for k in range(NK):
            nch = int(nchgk[g, k])
            lst = []
            blks = list(range(g * SB, min((g + 1) * SB, NB)))
            tstart = int(t0_bk[blks[0], k]) if nch else 0
            for j0 in range(0, nch, SUBCH):
                nsub = min(SUBCH, nch - j0)
                lst.append((int(icol0[g, k]) + j0 * 8, nsub, tstart + j0))
            row.append(tuple(lst))
        subs.append(tuple(row))

    # per-block matmul op list (k, sub-index, chunk-offset-in-sub, global chunk)
    blk_ops = []
    for b in range(NB):
        g = b // SB
        blks = range(g * SB, min((g + 1) * SB, NB))
        ops = []
        for k in range(NK):
            cc0 = sum(int(nch_bk[bb, k]) for bb in blks if bb < b)
            for j in range(int(nch_bk[b, k])):
                cc = cc0 + j
                ops.append((k, cc // SUBCH, cc % SUBCH, int(t0_bk[b, k]) + j))
        blk_ops.append(tuple(ops))

    meta = Meta(N=N, F=F, H=H, G=G, L=L, C=C, NS=NS, NB=NB, NK=NK, NG=NG,
                SB=SB, kbase=kbase, klen=klen,
                nchgk=tuple(tuple(int(v) for v in row) for row in nchgk),
                subs=tuple(subs),
                blk_ops=tuple(blk_ops), T_tot=T_tot, ICOLS=ICOLS)

    # per-core start offsets into the sorted edge array for (block, bucket)
    starts = np.zeros(C * NB * NK, np.int64)
    cnt_cbk = np.transpose(cnt, (0, 1, 2))  # already [C, NB, NK]
    # lexsort order is core, group, bucket, block -- NOT core, block, bucket.
    # Build per-(c,g,k,b) segment starts by walking in that order.
    seg_sizes = np.zeros((C, NG, NK, SB), np.int64)
    for c in range(C):
        for g in range(NG):
            blks = list(range(g * SB, min((g + 1) * SB, NB)))
            for ki in range(NK):
                for bi, b in enumerate(blks):
                    seg_sizes[c, g, ki, bi] = cnt[c, b, ki]
    flat_sizes = seg_sizes.ravel()
    flat_starts = np.zeros_like(flat_sizes)
    flat_starts[1:] = np.cumsum(flat_sizes)[:-1]
    seg_starts = flat_starts.reshape(C, NG, NK, SB)

    xT = np.ascontiguousarray(x.T)  # [F, N] f32

    iota128 = np.tile(np.arange(128, dtype=np.float32), (128, 1))
    iota_part = np.arange(128, dtype=np.float32).reshape(128, 1)
    iotaG = np.tile(np.arange(G, dtype=np.float32), (128, 1))
    ident_f = np.eye(128, dtype=np.float32)

    cntg = np.bincount(batch, minlength=G).astype(np.float32)
    invc = np.tile((1.0 / np.maximum(cntg, 1.0))[None, :], (H, 1)).astype(np.float32)

    conv_W = np.asarray(conv_W, np.float32)
    conv_b = np.asarray(conv_b, np.float32)

    in_maps = []
    for c in range(C):
        base = c * NS
        idx_img = np.zeros((128, ICOLS), np.int16)
        dstloc = np.full((128, T_tot), -1.0, np.float32)
        dinvdst = np.zeros((128, T_tot), np.float32)
        for g in range(NG):
            blks = list(range(g * SB, min((g + 1) * SB, NB)))
            for k in range(NK):
                nch = int(nchgk[g, k])
                if nch == 0:
                    continue
                nidx = nch * 128
                srel = np.zeros(nidx, np.int64)
                dl = np.full(nidx, -1.0, np.float32)
                dw = np.zeros(nidx, np.float32)
                pos = 0
                for bi, b in enumerate(blks):
                    n = int(cnt[c, b, k])
                    s0 = int(seg_starts[c, g, k, bi])
                    if n > 0:
                        e_src = src_s[s0:s0 + n]
                        e_dst = dst_s[s0:s0 + n]
                        srel[pos:pos + n] = e_src - kbase[k]
                        dl[pos:pos + n] = (e_dst - base - b * 128)
                        dw[pos:pos + n] = dinv[e_dst]
                    pos0 = pos
                    pos += int(nch_bk[b, k]) * 128
                    # pad rows within this block's chunk range gather row 0
                    # (valid; one-hot row is all-zero via dstloc=-1)
                assert pos <= nidx
                assert pos == nidx, (pos, nidx)
                # idx image: j -> [p = j%16 (replicated x8), col = j//16]
                arr = srel.reshape(-1, 16).T.astype(np.int16)  # [16, ncol]
                ic0 = int(icol0[g, k])
                idx_img[:, ic0:ic0 + nch * 8] = np.tile(arr, (8, 1))
                # dstloc/dinvdst: chunk-major columns
                t0 = None
                # global chunks for this (g,k) span consecutive t from
                # t0_bk[blks[0], k] in block order
                dl2 = dl.reshape(nch, 128).T      # [128, nch]
                dw2 = dw.reshape(nch, 128).T
                tstart = int(t0_bk[blks[0], k])
                dstloc[:, tstart:tstart + nch] = dl2
                dinvdst[:, tstart:tstart + nch] = dw2

        dinv_sh = dinv[base:base + NS]
        dinv_nm = np.zeros((128, NB), np.float32)
        poolid = np.full((128, NB), -1.0, np.float32)
        for b in range(NB):
            w = min(128, NS - b * 128)
            dinv_nm[:w, b] = dinv_sh[b * 128:b * 128 + w]
            poolid[:w, b] = batch[base + b * 128: base + b * 128 + w]
        m = {
            "x_t": np.ascontiguousarray(xT[:, base:base + NS]).astype(BF),
            "idx_img": idx_img,
            "dstloc": dstloc,
            "dinvdst": dinvdst,
            "poolid": poolid,
            "dinv_nm": dinv_nm,
            "iota128": iota128.astype(BF),
            "iota_part": iota_part,
            "iotag": iotaG.astype(BF),
            "ident_f": ident_f,
            "wemb": np.asarray(W_emb, np.float32).astype(BF),
            "bemb": np.asarray(b_emb, np.float32).reshape(H, 1),
            "invc": invc,
            "w1": np.asarray(W1, np.float32),
            "b1": np.asarray(b1, np.float32).reshape(-1, 1),
            "w2": np.asarray(W2, np.float32),
            "b2": np.asarray(b2, np.float32).reshape(-1, 1),
            "w3": np.asarray(W3, np.float32),
            "b3": np.asarray(b3, np.float32).reshape(1, 1),
        }
        for i in range(L):
            m[f"cw{i}"] = np.ascontiguousarray(conv_W[i])
            m[f"cb{i}"] = conv_b[i].reshape(H, 1)
        in_maps.append(m)
    return meta, in_maps


def build_nc(meta: Meta, gbufs=10, ohbufs=24, repeats=1,
             dbg_L=None, dbg_ng=None):
    N, F, H, G, L, C = meta.N, meta.F, meta.H, meta.G, meta.L, meta.C
    NS, NB, CH = meta.NS, meta.NB, meta.CH
    NK, NG, SB = meta.NK, meta.NG, meta.SB
    kbase, klen = meta.kbase, meta.klen
    nchgk, subs, blk_ops = meta.nchgk, meta.subs, meta.blk_ops
    T_tot, ICOLS = meta.T_tot, meta.ICOLS
    NCH = (NS + CH - 1) // CH

    nc = bacc.Bacc("TRN2", target_bir_lowering=False, debug=False, num_devices=C,
                   num_swdge_queues=4)

    def EIN(name, shape, dt):
        return nc.dram_tensor(name, list(shape), dt, kind="ExternalInput")

    x_t = EIN("x_t", [F, NS], BF16)
    idx_img = EIN("idx_img", [128, ICOLS], I16)
    dstloc = EIN("dstloc", [128, T_tot], F32)
    dinvdst = EIN("dinvdst", [128, T_tot], F32)
    poolid = EIN("poolid", [128, NB], F32)
    dinv_nm = EIN("dinv_nm", [128, NB], F32)
    iota128 = EIN("iota128", [128, 128], BF16)
    iota_part = EIN("iota_part", [128, 1], F32)
    iotag = EIN("iotag", [128, G], BF16)
    ident_f = EIN("ident_f", [128, 128], F32)
    wemb = EIN("wemb", [F, H], BF16)
    bemb = EIN("bemb", [H, 1], F32)
    invc = EIN("invc", [H, G], F32)
    w1 = EIN("w1", [H, H], F32)
    b1 = EIN("b1", [H, 1], F32)
    w2 = EIN("w2", [H, H // 2], F32)
    b2 = EIN("b2", [H // 2, 1], F32)
    w3 = EIN("w3", [H // 2, 1], F32)
    b3 = EIN("b3", [1, 1], F32)
    cw = [EIN(f"cw{i}", [H, H], F32) for i in range(L)]
    cb = [EIN(f"cb{i}", [H, 1], F32) for i in range(L)]

    out_d = nc.dram_tensor("out", [1, G], F32, kind="ExternalOutput")

    table_a = nc.dram_tensor("table_a", [N, 128], BF16, addr_space="Shared")
    table_b = nc.dram_tensor("table_b", [N, 128], BF16, addr_space="Shared")
    bounce = nc.dram_tensor("bounce", [NS, 128], BF16)
    pool_in = nc.dram_tensor("pool_in", [H, G], F32)
    pool_out = nc.dram_tensor("pool_out", [H, G], F32, addr_space="Shared")

    groups = [list(range(C))]

    with tile.TileContext(nc) as tc:
        import contextlib
        ctx = contextlib.ExitStack()
        with ctx:
            P = ctx.enter_context
            persist = P(tc.tile_pool(name="persist", bufs=1))
            xpool = P(tc.tile_pool(name="xpool", bufs=3))
            gpool = P(tc.tile_pool(name="gpool", bufs=gbufs))
            ohpool = P(tc.tile_pool(name="ohpool", bufs=ohbufs))
            slpool = P(tc.tile_pool(name="slpool", bufs=4))
            stpool = P(tc.tile_pool(name="stpool", bufs=4))
            pohpool = P(tc.tile_pool(name="pohpool", bufs=3))
            bp_ps = P(tc.tile_pool(name="bp_ps", bufs=3, space="PSUM"))
            mm_ps = P(tc.tile_pool(name="mm_ps", bufs=2, space="PSUM"))
            tr_ps = P(tc.tile_pool(name="tr_ps", bufs=2, space="PSUM"))

            nc.gpsimd.load_library(library_config.mlp)

            def load(name, ap, shape, dt):
                t = persist.tile(list(shape), dt, tag=name, name=name)
                nc.sync.dma_start(out=t[:], in_=ap[:])
                return t

            idx_sb = load("idx_sb", idx_img, [128, ICOLS], I16)
            dstloc_sb = load("dstloc_sb", dstloc, [128, T_tot], F32)
            dinvdst_sb = load("dinvdst_sb", dinvdst, [128, T_tot], F32)
            poolid_sb = load("poolid_sb", poolid, [128, NB], F32)
            dinvnm_sb = load("dinvnm_sb", dinv_nm, [128, NB], F32)
            iota_sb = load("iota_sb", iota128, [128, 128], BF16)
            iotap_sb = load("iotap_sb", iota_part, [128, 1], F32)
            iotag_sb = load("iotag_sb", iotag, [128, G], BF16)
            identf_sb = load("identf_sb", ident_f, [128, 128], F32)
            wemb_sb = load("wemb_sb", wemb, [F, H], BF16)
            bemb_sb = load("bemb_sb", bemb, [H, 1], F32)
            invc_sb = load("invc_sb", invc, [H, G], F32)
            w1_sb = load("w1_sb", w1, [H, H], F32)
            b1_sb = load("b1_sb", b1, [H, 1], F32)
            w2_sb = load("w2_sb", w2, [H, H // 2], F32)
            b2_sb = load("b2_sb", b2, [H // 2, 1], F32)
            w3_sb = load("w3_sb", w3, [H // 2, 1], F32)
            b3_sb = load("b3_sb", b3, [1, 1], F32)
            cw_sb = [load(f"cw{i}_sb", cw[i], [H, H], F32) for i in range(L)]
            cb_sb = [load(f"cb{i}_sb", cb[i], [H, 1], F32) for i in range(L)]

            hagg = persist.tile([64, NS], F32, tag="hagg")
            h3n = persist.tile([128, NB * 64], BF16, tag="h3n")

            def chunks():
                for ci in range(NCH):
                    c0 = ci * CH
                    yield c0, min(CH, NS - c0)

            def table_write(table_out):
                for b in range(NB):
                    w = min(128, NS - b * 128)
                    tp = tr_ps.tile([128, 64], F32, tag="trp")
                    nc.tensor.transpose(
                        out=tp[:w, :], in_=hagg[:, b * 128:b * 128 + w],
                        identity=identf_sb[:64, :64])
                    st = stpool.tile([128, 128], BF16, tag="st")
                    nc.vector.tensor_scalar(
                        out=st[:w, :64], in0=tp[:w, :],
                        scalar1=dinvnm_sb[:w, b:b + 1], scalar2=None,
                        op0=mybir.AluOpType.mult)
                    nc.sync.dma_start(
                        out=bounce[b * 128:b * 128 + w, :], in_=st[:w, :])
                nc.gpsimd.collective_compute(
                    "AllGather", mybir.AluOpType.bypass,
                    replica_groups=groups,
                    ins=[bounce[:]], outs=[table_out[:]])

            for _rep in range(repeats):
                # ================= embed =================
                for c0, cwd in chunks():
                    xt = xpool.tile([F, CH], BF16, tag="xt")
                    nc.sync.dma_start(out=xt[:, :cwd], in_=x_t[:, c0:c0 + cwd])
                    ps = mm_ps.tile([64, CH], F32, tag="mmps")
                    nc.tensor.matmul(out=ps[:, :cwd], lhsT=wemb_sb[:], rhs=xt[:, :cwd],
                                     start=True, stop=True)
                    nc.scalar.activation(out=hagg[:, c0:c0 + cwd], in_=ps[:, :cwd],
                                         func=mybir.ActivationFunctionType.Relu,
                                         bias=bemb_sb[:, 0:1])
                table_write(table_a)

                # ================= conv layers =================
                tables = [table_a, table_b, table_a]
                n_layers = L if dbg_L is None else dbg_L
                qrr = [0]
                for li in range(n_layers):
                    t_in = tables[li]
                    for g in range(NG if dbg_ng is None else dbg_ng):
                        gts = {}
                        for k in range(NK):
                            for j, (icol, nsub, _ts) in enumerate(subs[g][k]):
                                gt = gpool.tile([128, 24, 128], BF16, tag="g")
                                nidx = nsub * 128
                                nc.gpsimd.dma_gather(
                                    gt[:, :nsub, :],
                                    t_in[kbase[k]:kbase[k] + klen[k], :],
                                    idx_sb[:, icol:icol + nsub * 8],
                                    nidx, nidx, 128,
                                    # packed descriptors (16 rows/desc) keep
                                    # ring usage low; 4 SWDGE queues give the
                                    # DMA row-processing parallelism
                                    single_packet=False,
                                    queue_num=qrr[0] & 3)
                                qrr[0] += 1
                                gts[(k, j)] = gt
                        for b in range(g * SB, min((g + 1) * SB, NB)):
                            w = min(128, NS - b * 128)
                            ps = bp_ps.tile([64, 128], F32, tag="bps")
                            first = True
                            for (k, j, cc, t) in blk_ops[b]:
                                oh = ohpool.tile([128, 128], BF16, tag="oh")
                                nc.vector.tensor_scalar(
                                    out=oh[:], in0=iota_sb[:],
                                    scalar1=dstloc_sb[:, t:t + 1],
                                    scalar2=dinvdst_sb[:, t:t + 1],
                                    op0=mybir.AluOpType.is_equal,
                                    op1=mybir.AluOpType.mult)
                                nc.tensor.matmul(
                                    out=ps[:], lhsT=gts[(k, j)][:, cc, :64], rhs=oh[:],
                                    start=first, stop=False)
                                first = False
                            # self-loop tile from bounce (own shard rows)
                            gs = slpool.tile([128, 128], BF16, tag="gs")
                            nc.sync.dma_start(
                                out=gs[:w, :], in_=bounce[b * 128:b * 128 + w, :])
                            ohs = ohpool.tile([128, 128], BF16, tag="oh")
                            nc.vector.tensor_scalar(
                                out=ohs[:w, :], in0=iota_sb[:w, :],
                                scalar1=iotap_sb[:w, 0:1],
                                scalar2=dinvnm_sb[:w, b:b + 1],
                                op0=mybir.AluOpType.is_equal,
                                op1=mybir.AluOpType.mult)
                            nc.tensor.matmul(
                                out=ps[:], lhsT=gs[:w, :64], rhs=ohs[:w, :],
                                start=False, stop=True)
                            nc.vector.tensor_copy(
                                out=hagg[:, b * 128:b * 128 + w], in_=ps[:, :w])
                    for c0, cwd in chunks():
                        ps = mm_ps.tile([64, CH], F32, tag="mmps")
                        nc.tensor.matmul(out=ps[:, :cwd], lhsT=cw_sb[li][:],
                                         rhs=hagg[:, c0:c0 + cwd], start=True, stop=True)
                        nc.scalar.activation(out=hagg[:, c0:c0 + cwd], in_=ps[:, :cwd],
                                             func=mybir.ActivationFunctionType.Relu,
                                             bias=cb_sb[li][:, 0:1])
                    if li < L - 1:
                        table_write(tables[li + 1])

                # ================= pooling =================
                for b in range(NB):
                    w = min(128, NS - b * 128)
                    tp = tr_ps.tile([128, 64], F32, tag="trp")
                    nc.tensor.transpose(out=tp[:w, :], in_=hagg[:, b * 128:b * 128 + w],
                                        identity=identf_sb[:64, :64])
                    nc.vector.tensor_copy(out=h3n[:w, b * 64:(b + 1) * 64], in_=tp[:w, :])
                with tc.tile_pool(name="pool_ps", bufs=1, space="PSUM") as pool_ps:
                    pps = pool_ps.tile([64, G], F32, tag="pps")
                    for b in range(NB):
                        w = min(128, NS - b * 128)
                        ohp = pohpool.tile([128, G], BF16, tag="ohp")
                        nc.vector.tensor_scalar(
                            out=ohp[:w, :], in0=iotag_sb[:w, :],
                            scalar1=poolid_sb[:w, b:b + 1], scalar2=None,
                            op0=mybir.AluOpType.is_equal)
                        nc.tensor.matmul(out=pps[:], lhsT=h3n[:w, b * 64:(b + 1) * 64],
                                         rhs=ohp[:w, :], start=(b == 0), stop=(b == NB - 1))
                    psum_sb = persist.tile([64, G], F32, tag="psum_sb")
                    nc.vector.tensor_copy(out=psum_sb[:], in_=pps[:])
                nc.sync.dma_start(out=pool_in[:], in_=psum_sb[:])
                nc.gpsimd.collective_compute(
                    "AllReduce", mybir.AluOpType.add, replica_groups=groups,
                    ins=[pool_in[:]], outs=[pool_out[:]])
                pooled = persist.tile([64, G], F32, tag="pooled")
                nc.sync.dma_start(out=pooled[:], in_=pool_out[:])
                nc.vector.tensor_tensor(out=pooled[:], in0=pooled[:], in1=invc_sb[:],
                                        op=mybir.AluOpType.mult)
                # ================= MLP =================
                ps1 = mm_ps.tile([64, CH], F32, tag="mmps")
                nc.tensor.matmul(out=ps1[:, :G], lhsT=w1_sb[:], rhs=pooled[:],
                                 start=True, stop=True)
                r1 = persist.tile([64, G], F32, tag="r1")
                nc.scalar.activation(out=r1[:], in_=ps1[:64, :G],
                                     func=mybir.ActivationFunctionType.Relu,
                                     bias=b1_sb[:, 0:1])
                ps2 = mm_ps.tile([64, CH], F32, tag="mmps")
                nc.tensor.matmul(out=ps2[:32, :G], lhsT=w2_sb[:], rhs=r1[:],
                                 start=True, stop=True)
                r2 = persist.tile([32, G], F32, tag="r2")
                nc.scalar.activation(out=r2[:], in_=ps2[:32, :G],
                                     func=mybir.ActivationFunctionType.Relu,
                                     bias=b2_sb[:, 0:1])
                ps3 = mm_ps.tile([64, CH], F32, tag="mmps")
                nc.tensor.matmul(out=ps3[:1, :G], lhsT=w3_sb[:], rhs=r2[:],
                                 start=True, stop=True)
                outs = persist.tile([1, G], F32, tag="outs")
                nc.vector.tensor_scalar(out=outs[:], in0=ps3[:1, :G],
                                        scalar1=b3_sb[0:1, 0:1], scalar2=None,
                                        op0=mybir.AluOpType.add)
                nc.sync.dma_start(out=out_d[:], in_=outs[:])

    nc.compile()
    return nc


class SpmdRunner:
    def __init__(self, nc, n_cores):
        install_neuronx_cc_hook()
        self.nc = nc
        self.n_cores = n_cores
        partition_name = (nc.partition_id_tensor.name
                          if nc.partition_id_tensor else None)
        in_names, out_names, out_avals, zero_outs = [], [], [], []
        for alloc in nc.m.functions[0].allocations:
            if not isinstance(alloc, mybir.MemoryLocationSet):
                continue
            name = alloc.memorylocations[0].name
            if alloc.kind == "ExternalInput":
                if name != partition_name:
                    in_names.append(name)
            elif alloc.kind == "ExternalOutput":
                shape = tuple(alloc.tensor_shape)
                dt = mybir.dt.np(alloc.dtype)
                out_names.append(name)
                out_avals.append(jax.core.ShapedArray(shape, dt))
                zero_outs.append(np.zeros(shape, dt))
        self.in_names, self.out_names = in_names, out_names
        self.zero_outs = zero_outs
        bind_in_names = in_names + out_names
        if partition_name is not None:
            bind_in_names.append(partition_name)

        def _body(*args):
            operands = list(args)
            if partition_name is not None:
                operands.append(bass2jax.partition_id_tensor())
            outs = _bass_exec_p.bind(
                *operands,
                out_avals=tuple(out_avals),
                in_names=tuple(bind_in_names),
                out_names=tuple(out_names),
                lowering_input_output_aliases=(),
                sim_require_finite=False,
                sim_require_nnan=False,
                nc=nc,
            )
            return tuple(outs)

        devices = jax.devices()[:n_cores]
        self.mesh = Mesh(np.asarray(devices), ("core",))
        n_args = len(in_names) + len(zero_outs)
        in_specs = (PartitionSpec("core"),) * n_args
        out_specs = (PartitionSpec("core"),) * len(out_names)
        self.fn = jax.jit(
            shard_map(_body, mesh=self.mesh, in_specs=in_specs,
                      out_specs=out_specs, check_rep=False),
            keep_unused=True,
        )
        self._dev_in = None

    def set_inputs(self, in_maps):
        assert len(in_maps) == self.n_cores
        concat = [np.concatenate([np.asarray(in_maps[c][n])
                                  for c in range(self.n_cores)], axis=0)
                  for n in self.in_names]
        self._dev_in = [jax.device_put(a) for a in concat]
        self._dev_zeros = [
            jax.device_put(np.zeros((self.n_cores * z.shape[0], *z.shape[1:]),
                                    z.dtype)) for z in self.zero_outs]
        jax.block_until_ready(self._dev_in)

    def run(self):
        outs = self.fn(*self._dev_in, *self._dev_zeros)
        jax.block_until_ready(outs)
        return outs

    def results(self, outs):
        res = [dict() for _ in range(self.n_cores)]
        for i, name in enumerate(self.out_names):
            arr = np.asarray(outs[i])
            per = np.split(arr, self.n_cores, axis=0)
            for c in range(self.n_cores):
                res[c][name] = per[c]
        return res


_CACHE = {}


def _get_runner(meta, in_maps, repeats=1):
    key = (repeats, meta.T_tot, meta.ICOLS, meta.N, meta.L)
    if key not in _CACHE:
        nc = build_nc(meta, repeats=repeats)
        _CACHE[key] = SpmdRunner(nc, meta.C)
    return _CACHE[key]


def kernel(x, edge_index, batch, W_emb, b_emb, conv_W, conv_b,
           W1, b1, W2, b2, W3, b3):
    """Full (unsharded) inputs -> full [G, 1] float32 output."""
    G = 256
    meta, in_maps = preprocess(
        x, edge_index, batch, W_emb, b_emb, conv_W, conv_b,
        W1, b1, W2, b2, W3, b3, n_cores=8, G=G)
    r = _get_runner(meta, in_maps)
    r.set_inputs(in_maps)
    res = r.results(r.run())
    return np.ascontiguousarray(res[0]["out"].reshape(G, 1).astype(np.float32))
